# revision 4
# baseline (speedup 1.0000x reference)
"""Trainium2 Bass kernel for FlowNetC-style Correlation.

Problem: inputs [8, 256, 64, 128] f32 x2 -> output [8, 441, 64, 128] f32.
out[b, k, y, x] = mean_c in1[b,c,y,x] * pad(in2)[b, c, y+sy, x+sx],
with (sy, sx) = 2*(k//21, k%21), pad = 20 on each spatial side.

Strategy (per core = one batch element, data-parallel over B=8):
  The per-position channel dot-products are computed on the TensorEngine as a
  *blocked* band matmul: stationary = bf16 in1 block of 128 columns
  (16 y-values x 8 x-values, one (y,x)-parity), moving = bf16 zero-padded in2
  window (36 y' x 28 x' = 1008 columns, same parity), contracting over C=256
  (2 chunks of 128 partitions).  Every PSUM cell (m=(yi,xi), n=(vi,ui)) whose
  displacement (vi-yi, ui-xi) lands in [0,20]^2 is a distinct output element;
  the rest is benign overcompute (~2.3x).  The device scales by 1/C, casts to
  bf16 and dumps the raw band to DRAM; the host extracts the valid diagonal
  cells with a zero-copy strided view.
"""

import os
import sys

import numpy as np
import ml_dtypes

for _p in ("/opt/trn_rl_repo",):
    if _p not in sys.path:
        sys.path.insert(0, _p)

# ---- problem constants (hardcoded per contract) ----
B, C, H, W = 8, 256, 64, 128
PAD = 20
HP, WP = H + 2 * PAD, W + 2 * PAD          # 104, 168
P_, R_ = 16, 8                              # yi, xi block sizes (reduced coords)
VI, UI = 36, 28                             # moving window (reduced coords)
NOFF = 21                                   # displacements per axis
NCORES = 8

_cache = {}


def _build(n_cores: int):
    import concourse.tile as tile
    from concourse import bacc, mybir

    nc = bacc.Bacc(
        "TRN2", target_bir_lowering=False, debug=False, num_devices=n_cores
    )
    f32 = mybir.dt.float32
    bf16 = mybir.dt.bfloat16

    in1_d = nc.dram_tensor("in1", (C, H, W), f32, kind="ExternalInput")
    in2_d = nc.dram_tensor("in2", (C, H, W), f32, kind="ExternalInput")
    band_d = nc.dram_tensor(
        "band", (64, 128, 2, 504), bf16, kind="ExternalOutput"
    )

    with tile.TileContext(nc) as tc:
        with (
            tc.tile_pool(name="const", bufs=1) as cpool,
            tc.tile_pool(name="band", bufs=4) as bpool,
            tc.tile_pool(name="psum", bufs=3, space="PSUM") as ppool,
        ):
            A_sb = cpool.tile([128, 2, H, W], bf16)
            A_blk = cpool.tile([128, 2, 64, 128], bf16)
            B_sb = cpool.tile([128, 2, HP, WP], bf16)
            # zero-fill so the padded border multiplies to zero contributions
            nc.vector.memset(B_sb[:], 0.0)

            # input loads, f32 DRAM -> bf16 SBUF (cast => SWDGE / gpsimd)
            for ch in range(2):
                cs = slice(ch * 128, (ch + 1) * 128)
                for yq in range(4):
                    ys = slice(yq * 16, (yq + 1) * 16)
                    nc.gpsimd.dma_start(
                        A_sb[:, ch, ys, :], in1_d[cs, ys, :]
                    )
                    nc.gpsimd.dma_start(
                        B_sb[:, ch, PAD + yq * 16 : PAD + (yq + 1) * 16,
                             PAD : PAD + W],
                        in2_d[cs, ys, :],
                    )

            # Pre-block the stationary operand: the tensor engine's weights AP
            # allows only one free dimension, so gather each 128-column block
            # (16 y x 8 x, strided parity picks) into contiguous form.
            for ch in range(2):
                blk = 0
                for py in range(2):
                    for px in range(2):
                        for t in range(2):
                            ybase = py + 32 * t
                            for xb in range(8):
                                xbase = px + 16 * xb
                                src = A_sb[:, ch,
                                           ybase : min(ybase + 32, H) : 2,
                                           xbase : min(xbase + 16, W) : 2]
                                dst = A_blk[:, ch, blk, :].rearrange(
                                    "p (a b) -> p a b", a=P_
                                )
                                if blk % 2 == 0:
                                    nc.vector.tensor_copy(dst, src)
                                else:
                                    nc.scalar.copy(dst, src)
                                blk += 1

            pair = 0
            for py in range(2):
                for px in range(2):
                    for t in range(2):
                        ybase = py + 32 * t
                        for xb in range(8):
                            xbase = px + 16 * xb
                            ps = ppool.tile([128, 2, 512], f32)
                            for ch in range(2):
                                lhsT = A_blk[:, ch, pair, :]
                                for vh in range(2):
                                    yb2 = ybase + 36 * vh
                                    rhs = B_sb[:, ch,
                                               yb2 : min(yb2 + 36, HP) : 2,
                                               xbase : min(xbase + 56, WP) : 2]
                                    nc.tensor.matmul(
                                        ps[:, vh, 0:504],
                                        lhsT,
                                        rhs,
                                        start=(ch == 0),
                                        stop=(ch == 1),
                                    )
                            bt = bpool.tile([128, 2, 504], bf16)
                            if pair % 2 == 0:
                                nc.vector.tensor_scalar_mul(
                                    bt[:], ps[:, :, 0:504], 1.0 / C
                                )
                            else:
                                nc.scalar.mul(bt[:], ps[:, :, 0:504], 1.0 / C)
                            nc.sync.dma_start(band_d[pair], bt[:])
                            pair += 1

    nc.compile()
    return nc


def _get_nc(n_cores: int):
    key = ("nc", n_cores)
    if key not in _cache:
        _cache[key] = _build(n_cores)
    return _cache[key]


def _extract(band: np.ndarray) -> np.ndarray:
    """band [64, 128, 2, 504] bf16 for one batch -> out [441, H, W] f32."""
    arr = np.ascontiguousarray(band).reshape(2, 2, 2, 8, P_, R_, VI, UI)
    s = arr.strides
    D = np.lib.stride_tricks.as_strided(
        arr,
        shape=(2, 2, 2, 8, P_, R_, NOFF, NOFF),
        strides=(s[0], s[1], s[2], s[3], s[4] + s[6], s[5] + s[7], s[6], s[7]),
    )
    out = np.empty((NOFF * NOFF, H, W), np.float32)
    out8 = out.reshape(NOFF, NOFF, 2, P_, 2, 8, R_, 2)
    # D dims: (py,px,t,xb,yi,xi,dy,dx) -> out dims (dy,dx,t,yi,py,xb,xi,px)
    out8[:] = np.transpose(D, (6, 7, 2, 4, 0, 3, 5, 1)).astype(np.float32)
    return out


def kernel(input1: np.ndarray, input2: np.ndarray) -> np.ndarray:
    from concourse import bass_utils

    in1 = np.ascontiguousarray(np.asarray(input1), dtype=np.float32)
    in2 = np.ascontiguousarray(np.asarray(input2), dtype=np.float32)
    assert in1.shape == (B, C, H, W) and in2.shape == (B, C, H, W)

    nc = _get_nc(NCORES)
    in_maps = [{"in1": in1[b], "in2": in2[b]} for b in range(B)]
    trace = bool(int(os.environ.get("CORR_TRACE", "0")))
    res = bass_utils.run_bass_kernel_spmd(
        nc, in_maps, core_ids=list(range(NCORES)), trace=trace
    )
    _cache["last_exec_time_ns"] = res.exec_time_ns

    out = np.empty((B, NOFF * NOFF, H, W), np.float32)
    for b in range(B):
        out[b] = _extract(np.asarray(res.results[b]["band"]))
    return out


# revision 5
# speedup vs baseline: 1.4043x; 1.4043x over previous
"""Trainium2 Bass kernel for FlowNetC-style Correlation.

Problem: inputs [8, 256, 64, 128] f32 x2 -> output [8, 441, 64, 128] f32.
out[b, k, y, x] = mean_c in1[b,c,y,x] * pad(in2)[b, c, y+sy, x+sx],
with (sy, sx) = 2*(k//21, k%21), pad = 20 on each spatial side.

Strategy (per core = one batch element, data-parallel over B=8):
  The per-position channel dot products run on the TensorEngine as a *blocked*
  band matmul: stationary = bf16 in1 block of 128 columns (16 y-values x 8
  x-values, one (y,x)-parity), moving = bf16 in2 window (clipped to in-bounds
  rows/cols), contracting over C=256 (2 chunks of 128 partitions).  Every PSUM
  cell (m=(yi,xi), n=(vi,ui)) whose displacement (vi-yi, ui-xi) lands in
  [0,20]^2 is a distinct output element; the rest is benign overcompute.
  Out-of-bounds window positions yield exactly-zero outputs, so they are never
  computed: the host reconstructs them as zeros.  The device scales by 1/C,
  casts to bf16 and dumps the compacted band to DRAM; the host extracts the
  valid diagonal cells with a zero-copy strided view.
"""

import os
import sys

import numpy as np

for _p in ("/opt/trn_rl_repo",):
    if _p not in sys.path:
        sys.path.insert(0, _p)

# ---- problem constants (hardcoded per contract) ----
B, C, H, W = 8, 256, 64, 128
PAD = 20
P_, R_ = 16, 8                              # yi, xi block sizes (reduced coords)
VI, UI = 36, 28                             # full moving window (reduced coords)
NOFF = 21                                   # displacements per axis
NCORES = 8

# clipped (in-bounds) moving-window ranges, precomputed per block class
UI_LO = [10, 2, 0, 0, 0, 0, 0, 0]           # by xb
UI_V = [18, 26, 28, 28, 28, 28, 26, 18]     # by xb
VI_LO = [10, 0]                             # by t  (vi count is 26 for both)

_cache = {}


def _build(n_cores: int):
    import concourse.tile as tile
    from concourse import bacc, mybir

    nc = bacc.Bacc(
        "TRN2", target_bir_lowering=False, debug=False, num_devices=n_cores
    )
    f32 = mybir.dt.float32
    bf16 = mybir.dt.bfloat16

    in1_d = nc.dram_tensor("in1", (C, H, W), f32, kind="ExternalInput")
    in2_d = nc.dram_tensor("in2", (C, H, W), f32, kind="ExternalInput")
    band_d = nc.dram_tensor(
        "band", (64, 128, 2, 13, 28), bf16, kind="ExternalOutput"
    )

    with tile.TileContext(nc) as tc:
        with (
            tc.tile_pool(name="const", bufs=1) as cpool,
            tc.tile_pool(name="band", bufs=4) as bpool,
            tc.tile_pool(name="psum", bufs=4, space="PSUM") as ppool,
        ):
            A_sb = cpool.tile([128, 2, H, W], bf16)
            A_blk = cpool.tile([128, 2, 64, 128], bf16)
            B_sb = cpool.tile([128, 2, H, W], bf16)

            # f32 DRAM -> bf16 SBUF loads (cast => SWDGE / gpsimd), ordered so
            # t=0 work can start while the rest streams in.
            def load_A(half):
                ys = slice(half * 32, (half + 1) * 32)
                for ch in range(2):
                    cs = slice(ch * 128, (ch + 1) * 128)
                    nc.gpsimd.dma_start(A_sb[:, ch, ys, :], in1_d[cs, ys, :])

            def load_B(quarter):
                ys = slice(quarter * 16, (quarter + 1) * 16)
                for ch in range(2):
                    cs = slice(ch * 128, (ch + 1) * 128)
                    nc.gpsimd.dma_start(B_sb[:, ch, ys, :], in2_d[cs, ys, :])

            def rearrange_A(t):
                # Gather each stationary block (16 y x 8 x, strided parity
                # picks) into one contiguous 128-column: the tensor engine's
                # weights AP allows only a single free dimension.
                k = 0
                for ch in range(2):
                    blk = 32 * t
                    for py in range(2):
                        for px in range(2):
                            ybase = py + 32 * t
                            for xb in range(8):
                                xbase = px + 16 * xb
                                src = A_sb[:, ch,
                                           ybase : min(ybase + 32, H) : 2,
                                           xbase : min(xbase + 16, W) : 2]
                                dst = A_blk[:, ch, blk, :].rearrange(
                                    "p (a b) -> p a b", a=P_
                                )
                                if k % 2 == 0:
                                    nc.vector.tensor_copy(dst, src)
                                else:
                                    nc.scalar.copy(dst, src)
                                blk += 1
                                k += 1

            def do_pairs(t, pair0):
                pair = pair0
                for py in range(2):
                    for px in range(2):
                        for xb in range(8):
                            ui_lo, ui_v = UI_LO[xb], UI_V[xb]
                            vi_lo = VI_LO[t]
                            c0 = px + 16 * xb + 2 * ui_lo - 20
                            ps = ppool.tile([128, 2, 512], f32)
                            for ch in range(2):
                                lhsT = A_blk[:, ch, pair, :]
                                for vh in range(2):
                                    r0 = py + 32 * t + 2 * (vi_lo + 13 * vh) - 20
                                    rhs = B_sb[:, ch,
                                               r0 : min(r0 + 26, H) : 2,
                                               c0 : min(c0 + 2 * ui_v, W) : 2]
                                    nc.tensor.matmul(
                                        ps[:, vh, 0 : 13 * ui_v],
                                        lhsT,
                                        rhs,
                                        start=(ch == 0),
                                        stop=(ch == 1),
                                    )
                            bt = bpool.tile([128, 2, 13, 28], bf16)
                            src = ps[:, :, 0 : 13 * ui_v].rearrange(
                                "p c (a b) -> p c a b", a=13
                            )
                            if pair % 2 == 0:
                                nc.vector.tensor_scalar_mul(
                                    bt[:, :, :, 0:ui_v], src, 1.0 / C
                                )
                            else:
                                nc.scalar.mul(bt[:, :, :, 0:ui_v], src, 1.0 / C)
                            nc.sync.dma_start(band_d[pair], bt[:])
                            pair += 1

            load_A(0)
            load_B(0)
            load_B(1)
            rearrange_A(0)
            load_B(2)
            load_B(3)
            load_A(1)
            do_pairs(0, 0)
            rearrange_A(1)
            do_pairs(1, 32)

    nc.compile()
    return nc


def _get_nc(n_cores: int):
    key = ("nc", n_cores)
    if key not in _cache:
        _cache[key] = _build(n_cores)
    return _cache[key]


def _extract(band: np.ndarray) -> np.ndarray:
    """band [64, 128, 2, 13, 28] bf16 for one batch -> out [441, H, W] f32."""
    # device pair order: [t, py, px, xb]; merged (vh, vr) -> vi' (stride 28)
    arr = np.ascontiguousarray(band).reshape(2, 2, 2, 8, P_, R_, 26, 28)
    P9 = np.zeros((2, 2, 2, 8, P_, R_, VI, UI), np.float32)
    for t in range(2):
        for xb in range(8):
            ui_lo, ui_v = UI_LO[xb], UI_V[xb]
            vi_lo = VI_LO[t]
            P9[t, :, :, xb, :, :, vi_lo : vi_lo + 26, ui_lo : ui_lo + ui_v] = (
                arr[t, :, :, xb, :, :, :, :ui_v]
            )
    s = P9.strides
    D = np.lib.stride_tricks.as_strided(
        P9,
        shape=(2, 2, 2, 8, P_, R_, NOFF, NOFF),
        strides=(s[0], s[1], s[2], s[3], s[4] + s[6], s[5] + s[7], s[6], s[7]),
    )
    out = np.empty((NOFF * NOFF, H, W), np.float32)
    out8 = out.reshape(NOFF, NOFF, 2, P_, 2, 8, R_, 2)
    # D dims: (t,py,px,xb,yi,xi,dy,dx) -> out dims (dy,dx,t,yi,py,xb,xi,px)
    out8[:] = np.transpose(D, (6, 7, 0, 4, 1, 3, 5, 2))
    return out


def kernel(input1: np.ndarray, input2: np.ndarray) -> np.ndarray:
    from concourse import bass_utils

    in1 = np.ascontiguousarray(np.asarray(input1), dtype=np.float32)
    in2 = np.ascontiguousarray(np.asarray(input2), dtype=np.float32)
    assert in1.shape == (B, C, H, W) and in2.shape == (B, C, H, W)

    nc = _get_nc(NCORES)
    in_maps = [{"in1": in1[b], "in2": in2[b]} for b in range(B)]
    trace = bool(int(os.environ.get("CORR_TRACE", "0")))
    res = bass_utils.run_bass_kernel_spmd(
        nc, in_maps, core_ids=list(range(NCORES)), trace=trace
    )
    _cache["last_exec_time_ns"] = res.exec_time_ns

    out = np.empty((B, NOFF * NOFF, H, W), np.float32)
    for b in range(B):
        out[b] = _extract(np.asarray(res.results[b]["band"]))
    return out
